# revision 1
# baseline (speedup 1.0000x reference)
"""MiniBatchDiscrimination kernel for 8 TRN2 NeuronCores.

out = concat([x, f], axis=1) where
  act = (x @ W + b).reshape(B, K, D)
  f[i,k] = sum_j exp(-(sum_d |act[i,k,d]-act[j,k,d]| + (i==j)))

Strategy (per core c, owning batch rows i in [128c, 128c+128)):
  - GEMM: actT_local [250(kd), 128(i)] = W^T @ x_c^T   (x passed pre-transposed)
  - AllGather actT over 8 cores -> actT [250, 1024] on every core, cast fp16
  - per i: DVE tensor_scalar (op0=subtract per-partition scalar, op1=abs_max 0)
      -> DIFF [125, 1024] fp16 x2 partition-chunks        (4x DVE mode)
  - PE matmul vs stationary 0/1 "comb" matrix contracts d (5) within each k
      -> L1 [100(2 i's x 50k), 1024] fp32 in PSUM
  - ACT exp(scale=-1) with accum_out -> feature column (j-sum fused)
  - diagonal eps: computed exp(0)=1, true exp(-1): add (e^-1 - 1) constant.
Host concatenates x with gathered per-core features.
"""

import math
import numpy as np

import concourse.bass as bass
import concourse.tile as tile
from concourse import mybir
from concourse.bass_utils import run_bass_kernel_spmd
from concourse.vector_clock import ScopedClock, VectorClock

B, F, K, D = 1024, 2048, 50, 5
KD = K * D          # 250
NCORES = 8
IB = B // NCORES    # 128 rows per core
PC = 125            # partition chunk: 25 whole k's of 5 d's
NCH = F // 128      # 16 contraction chunks for the GEMM

f32 = mybir.dt.float32
f16 = mybir.dt.float16


def _patched_drain_and_barrier(self, tick_clock, wait_clock):
    # Walrus in this container rejects the stock tail drain ("Too many sync
    # wait commands"): spread the global-clock waits over one NOP per proc.
    nc = self.nc
    gc = tick_clock.global_clock
    n = len(gc)
    for p in range(n):
        if gc[p] == 0:
            continue
        vec = [0] * n
        vec[p] = gc[p]
        nop = nc.sync.nop(nofuse=True, hint=f"tail_wait_p{p}")
        wait_clock.add_sem_waits(nop.ins, ScopedClock({None: VectorClock(vec)}))
    nc.sync.drain()
    nc.all_engine_barrier()
    assert self.sems is not None
    popped = nc._tile_sem_poison_stack.pop()
    assert popped is self._sem_poison
    nc.clear_and_free_semaphores(list(self.sems.allocated().values()))
    nc.all_engine_barrier()


tile.TileContext._drain_and_barrier = _patched_drain_and_barrier

_ws_ctr = [0]


def _split_excess_waits(nc, max_waits=1):
    """Walrus here allows only one sync-wait per instruction; hoist the rest
    onto same-engine NOPs inserted immediately before (program order on the
    engine preserves semantics)."""
    import bass_rust as _br

    for fn in nc.m.functions:
        new_blocks = []
        for bb in fn.blocks:
            out = []
            changed = False
            for inst in bb.instructions:
                si = inst.sync_info
                if si is not None and len(si.on_wait) > max_waits:
                    waits = list(si.on_wait)
                    for w in waits[:-max_waits]:
                        _ws_ctr[0] += 1
                        nop = mybir.InstNoOp(
                            name=f"WSplit-{_ws_ctr[0]}", ins=[], outs=[])
                        nop.engine = inst.engine
                        nop.sync_info = mybir.SyncInfo(
                            on_wait=[w], on_update=[])
                        out.append(nop)
                    inst.sync_info = mybir.SyncInfo(
                        on_wait=waits[-max_waits:], on_update=list(si.on_update))
                    changed = True
                out.append(inst)
            if changed:
                bb2 = _br.BasicBlock(name=bb.name, instructions=out)
                if bb.IsExit is not None:
                    bb2.IsExit = bb.IsExit
                if bb.IsLoopEntry is not None:
                    bb2.IsLoopEntry = bb.IsLoopEntry
                if bb.IsPredicated is not None:
                    bb2.IsPredicated = bb.IsPredicated
                new_blocks.append(bb2)
            else:
                new_blocks.append(bb)
        fn.blocks = new_blocks


def _build(split_waits=True):
    nc = bass.Bass("TRN2", target_bir_lowering=False, debug=False,
                   num_devices=NCORES)
    xt_d = nc.dram_tensor("xt", [F, IB], f32, kind="ExternalInput").ap()
    w_d = nc.dram_tensor("w", [F, KD], f32, kind="ExternalInput").ap()
    b_d = nc.dram_tensor("bias", [KD], f32, kind="ExternalInput").ap()
    comb_d = [nc.dram_tensor(f"comb{h}", [PC, 64], f16, kind="ExternalInput").ap()
              for h in range(2)]
    feat_d = nc.dram_tensor("feat", [IB, K], f32, kind="ExternalOutput")

    sub = mybir.AluOpType.subtract
    band = mybir.AluOpType.bitwise_and
    Exp = mybir.ActivationFunctionType.Exp
    Abs = mybir.ActivationFunctionType.Abs
    Ident = mybir.ActivationFunctionType.Identity

    with tile.TileContext(nc, num_cores=NCORES) as tc:
        with (
            tc.tile_pool(name="gemm_in", bufs=1) as gemm_in,
            tc.tile_pool(name="gemm_ps", bufs=1, space="PSUM") as gemm_ps,
            tc.tile_pool(name="acts", bufs=1) as acts,
            tc.tile_pool(name="dif", bufs=3) as difp,
            tc.tile_pool(name="l1", bufs=3, space="PSUM") as l1p,
            tc.tile_pool(name="outp", bufs=1) as outp,
        ):
            # ---- load + cast inputs (per fchunk, so the GEMM overlaps) ----
            xt32 = gemm_in.tile([128, NCH, IB], f32)       # xT  [f%128, fchunk, i]
            w32 = gemm_in.tile([128, NCH, KD], f32)        # W   [f%128, fchunk, kd]
            QC = 4   # chunks per DMA quarter
            for q in range(NCH // QC):
                c0 = q * QC
                nc.sync.dma_start(
                    xt32[:, c0:c0 + QC, :],
                    bass.AP(xt_d.tensor, c0 * 128 * IB,
                            [[IB, 128], [128 * IB, QC], [1, IB]]))
                nc.sync.dma_start(
                    w32[:, c0:c0 + QC, :],
                    bass.AP(w_d.tensor, c0 * 128 * KD,
                            [[KD, 128], [128 * KD, QC], [1, KD]]))

            bias_sb = [gemm_in.tile([PC, 1], f32, tag=f"bias{h}", name=f"bias_sb{h}") for h in range(2)]
            for h in range(2):
                nc.sync.dma_start(
                    bias_sb[h][:], bass.AP(b_d.tensor, h * PC, [[1, PC], [0, 1]]))

            # ---- GEMM: actT_local [125,128] x2 = W^T @ x_c^T + b ----
            actT_sb = [acts.tile([PC, IB], f32, tag=f"actT{h}", name=f"actT_sb{h}") for h in range(2)]
            pss = [gemm_ps.tile([PC, IB], f32, tag=f"gps{h}", name=f"gps{h}")
                   for h in range(2)]
            for c in range(NCH):
                for h in range(2):
                    nc.tensor.matmul(
                        pss[h][:], w32[:, c, h * PC:(h + 1) * PC], xt32[:, c, :],
                        start=(c == 0), stop=(c == NCH - 1))
            for h in range(2):
                # PSUM -> SBUF with per-partition bias add
                nc.scalar.activation(actT_sb[h][:], pss[h][:], Ident,
                                     bias=bias_sb[h][:], scale=1.0)

            # local actT in fp16 (shipped through the AllGather) plus an fp32
            # copy of the fp16-rounded values: tensor_scalar scalars must be
            # fp32, and they must bit-match gat16 so the diagonal is 0.
            lact16 = [acts.tile([PC, IB], f16, tag=f"lact16_{h}", name=f"lact16_{h}") for h in range(2)]
            lact32 = [acts.tile([PC, IB], f32, tag=f"lact32_{h}", name=f"lact32_{h}") for h in range(2)]
            for h in range(2):
                nc.vector.tensor_copy(lact16[h][:], actT_sb[h][:])
                nc.vector.tensor_copy(lact32[h][:], lact16[h][:])

            # ---- AllGather actT (fp16) over the 8 cores ----
            ag_in = nc.dram_tensor("ag_in", [2 * PC, IB], f16).ap()
            ag_out = nc.dram_tensor("ag_out", [NCORES * 2 * PC, IB], f16).ap()
            for h in range(2):
                nc.sync.dma_start(ag_in[h * PC:(h + 1) * PC, :], lact16[h][:])
            nc.gpsimd.collective_compute(
                "AllGather", mybir.AluOpType.bypass,
                replica_groups=[list(range(NCORES))],
                ins=[ag_in[:].opt()],
                outs=[ag_out[:].opt()])

            # ---- gather the 8 blocks into [125, 1024] per half ----
            gat16 = [acts.tile([PC, B], f16, tag=f"gat16_{h}", name=f"gat16_{h}") for h in range(2)]
            for h in range(2):
                eng = nc.sync if h == 0 else nc.scalar
                eng.dma_start(
                    gat16[h][:],
                    bass.AP(ag_out.tensor, h * PC * IB,
                            [[IB, PC], [2 * PC * IB, NCORES], [1, IB]]))

            # ---- comb matrices [125, 64] (host-built 0/1 d-summing pattern;
            # the two halves accumulate into one PSUM group, rows k = 0..49;
            # columns 50..63 are zero so the pad rows are initialized)
            combs = [acts.tile([PC, 64], f16, tag=f"comb{h}", name=f"comb{h}") for h in range(2)]
            for h in range(2):
                nc.sync.dma_start(combs[h][:], comb_d[h][:, :])

            # rows: isub*64 + k; rows 50..63 and 114..127 are unused
            feats = outp.tile([128, IB // 2], f32)

            # ---- main loop: 2 i's per group ----
            # ~3/16 of units run fused on ACT (Abs(a_i - in)); the rest on
            # DVE (tensor_scalar subtract at 4x + int16 sign-clear AND).
            ACT_UNITS = {5, 10, 15}
            unit = 0
            for g in range(IB // 2):
                difs = [None, None]
                for isub in range(2):
                    il = 2 * g + isub
                    dt_ = difp.tile([PC, 2 * B], f16, tag=f"dif{isub}",
                                    name=f"dif{isub}_{g}")
                    use_act = (unit % 16) in ACT_UNITS
                    unit += 1
                    if use_act:
                        for h in range(2):
                            nc.scalar.activation(
                                dt_[:, h * B:(h + 1) * B], gat16[h][:], Abs,
                                bias=lact32[h][:, il:il + 1], scale=-1.0)
                    else:
                        for h in range(2):
                            nc.vector.tensor_scalar(
                                out=dt_[:, h * B:(h + 1) * B], in0=gat16[h][:],
                                scalar1=lact32[h][:, il:il + 1], scalar2=None,
                                op0=sub)
                        dti = dt_[:].bitcast(mybir.dt.int16)
                        nc.vector.tensor_scalar(
                            out=dti, in0=dti, scalar1=0x7FFF, scalar2=None,
                            op0=band)
                    difs[isub] = dt_
                l1 = l1p.tile([128, B], f32, tag="l1")
                for isub in range(2):
                    off = isub * 64
                    for jh in range(2):
                        js = slice(jh * 512, (jh + 1) * 512)
                        for h in range(2):
                            seg = h * B + jh * 512
                            nc.tensor.matmul(
                                l1[off:off + 64, js], combs[h][:],
                                difs[isub][:, seg:seg + 512],
                                start=(h == 0), stop=(h == 1))
                nc.scalar.activation(l1[:], l1[:], Exp, scale=-1.0,
                                     accum_out=feats[:, g:g + 1])

            # ---- diagonal eps correction + store ----
            featc = outp.tile([128, IB // 2], f32)
            nc.vector.tensor_scalar(
                out=featc[:], in0=feats[:], scalar1=math.exp(-1.0) - 1.0,
                scalar2=None, op0=mybir.AluOpType.add)
            for isub in range(2):
                nc.sync.dma_start(
                    bass.AP(feat_d, 50 * isub, [[1, 50], [2 * K, IB // 2]]),
                    featc[isub * 64:isub * 64 + 50, :])

    if split_waits:
        _split_excess_waits(nc)
    return nc


_CACHE = {}
TRACE = False


def kernel(x, weights, bias):
    x = np.ascontiguousarray(x, dtype=np.float32)
    weights = np.ascontiguousarray(weights, dtype=np.float32)
    bias = np.ascontiguousarray(bias, dtype=np.float32)

    if "nc" not in _CACHE:
        _CACHE["nc"] = _build()
    nc = _CACHE["nc"]

    xt = np.ascontiguousarray(x.T)  # [F, B]
    combs = []
    for h in range(2):
        c = np.zeros((PC, 64), dtype=np.float16)
        for p in range(PC):
            c[p, p // D + 25 * h] = 1.0
        combs.append(c)
    in_maps = []
    for c in range(NCORES):
        in_maps.append({
            "xt": np.ascontiguousarray(xt[:, c * IB:(c + 1) * IB]),
            "w": weights,
            "bias": bias,
            "comb0": combs[0],
            "comb1": combs[1],
        })
    res = run_bass_kernel_spmd(nc, in_maps, list(range(NCORES)), trace=TRACE)
    _CACHE["last_res"] = res
    feats = np.concatenate([res.results[c]["feat"] for c in range(NCORES)],
                           axis=0)  # [B, K]
    return np.concatenate([x, feats.astype(np.float32)], axis=1)



# revision 2
# speedup vs baseline: 1.0084x; 1.0084x over previous
"""MiniBatchDiscrimination kernel for 8 TRN2 NeuronCores.

out = concat([x, f], axis=1) where
  act = (x @ W + b).reshape(B, K, D)
  f[i,k] = sum_j exp(-(sum_d |act[i,k,d]-act[j,k,d]| + (i==j)))

Strategy (per core c, owning batch rows i in [128c, 128c+128)):
  - GEMM: actT_local [250(kd), 128(i)] = W^T @ x_c^T   (x passed pre-transposed)
  - AllGather actT over 8 cores -> actT [250, 1024] on every core, cast fp16
  - per i: DVE tensor_scalar (op0=subtract per-partition scalar, op1=abs_max 0)
      -> DIFF [125, 1024] fp16 x2 partition-chunks        (4x DVE mode)
  - PE matmul vs stationary 0/1 "comb" matrix contracts d (5) within each k
      -> L1 [100(2 i's x 50k), 1024] fp32 in PSUM
  - ACT exp(scale=-1) with accum_out -> feature column (j-sum fused)
  - diagonal eps: computed exp(0)=1, true exp(-1): add (e^-1 - 1) constant.
Host concatenates x with gathered per-core features.
"""

import math
import numpy as np

import concourse.bass as bass
import concourse.tile as tile
from concourse import mybir
from concourse.bass_utils import run_bass_kernel_spmd
from concourse.vector_clock import ScopedClock, VectorClock

B, F, K, D = 1024, 2048, 50, 5
KD = K * D          # 250
NCORES = 8
IB = B // NCORES    # 128 rows per core
PC = 125            # partition chunk: 25 whole k's of 5 d's
NCH = F // 128      # 16 contraction chunks for the GEMM

f32 = mybir.dt.float32
f16 = mybir.dt.float16


def _patched_drain_and_barrier(self, tick_clock, wait_clock):
    # Walrus in this container rejects the stock tail drain ("Too many sync
    # wait commands"): spread the global-clock waits over one NOP per proc.
    nc = self.nc
    gc = tick_clock.global_clock
    n = len(gc)
    for p in range(n):
        if gc[p] == 0:
            continue
        vec = [0] * n
        vec[p] = gc[p]
        nop = nc.sync.nop(nofuse=True, hint=f"tail_wait_p{p}")
        wait_clock.add_sem_waits(nop.ins, ScopedClock({None: VectorClock(vec)}))
    nc.sync.drain()
    nc.all_engine_barrier()
    assert self.sems is not None
    popped = nc._tile_sem_poison_stack.pop()
    assert popped is self._sem_poison
    nc.clear_and_free_semaphores(list(self.sems.allocated().values()))
    nc.all_engine_barrier()


tile.TileContext._drain_and_barrier = _patched_drain_and_barrier

_ws_ctr = [0]


def _split_excess_waits(nc, max_waits=1):
    """Walrus here allows only one sync-wait per instruction; hoist the rest
    onto same-engine NOPs inserted immediately before (program order on the
    engine preserves semantics)."""
    import bass_rust as _br

    for fn in nc.m.functions:
        new_blocks = []
        for bb in fn.blocks:
            out = []
            changed = False
            for inst in bb.instructions:
                si = inst.sync_info
                if si is not None and len(si.on_wait) > max_waits:
                    waits = list(si.on_wait)
                    for w in waits[:-max_waits]:
                        _ws_ctr[0] += 1
                        nop = mybir.InstNoOp(
                            name=f"WSplit-{_ws_ctr[0]}", ins=[], outs=[])
                        nop.engine = inst.engine
                        nop.sync_info = mybir.SyncInfo(
                            on_wait=[w], on_update=[])
                        out.append(nop)
                    inst.sync_info = mybir.SyncInfo(
                        on_wait=waits[-max_waits:], on_update=list(si.on_update))
                    changed = True
                out.append(inst)
            if changed:
                bb2 = _br.BasicBlock(name=bb.name, instructions=out)
                if bb.IsExit is not None:
                    bb2.IsExit = bb.IsExit
                if bb.IsLoopEntry is not None:
                    bb2.IsLoopEntry = bb.IsLoopEntry
                if bb.IsPredicated is not None:
                    bb2.IsPredicated = bb.IsPredicated
                new_blocks.append(bb2)
            else:
                new_blocks.append(bb)
        fn.blocks = new_blocks


def _build(split_waits=True):
    nc = bass.Bass("TRN2", target_bir_lowering=False, debug=False,
                   num_devices=NCORES)
    xt_d = nc.dram_tensor("xt", [F, IB], f32, kind="ExternalInput").ap()
    w_d = nc.dram_tensor("w", [F, KD], f32, kind="ExternalInput").ap()
    b_d = nc.dram_tensor("bias", [KD], f32, kind="ExternalInput").ap()
    comb_d = [nc.dram_tensor(f"comb{h}", [PC, 64], f16, kind="ExternalInput").ap()
              for h in range(2)]
    feat_d = nc.dram_tensor("feat", [IB, K], f32, kind="ExternalOutput")

    sub = mybir.AluOpType.subtract
    band = mybir.AluOpType.bitwise_and
    Exp = mybir.ActivationFunctionType.Exp
    Abs = mybir.ActivationFunctionType.Abs
    Ident = mybir.ActivationFunctionType.Identity

    with tile.TileContext(nc, num_cores=NCORES) as tc:
        with (
            tc.tile_pool(name="gemm_in", bufs=1) as gemm_in,
            tc.tile_pool(name="gemm_ps", bufs=1, space="PSUM") as gemm_ps,
            tc.tile_pool(name="acts", bufs=1) as acts,
            tc.tile_pool(name="dif", bufs=3) as difp,
            tc.tile_pool(name="l1", bufs=3, space="PSUM") as l1p,
            tc.tile_pool(name="outp", bufs=1) as outp,
        ):
            # ---- load + cast inputs (per fchunk, so the GEMM overlaps) ----
            xt32 = gemm_in.tile([128, NCH, IB], f32)       # xT  [f%128, fchunk, i]
            w32 = gemm_in.tile([128, NCH, KD], f32)        # W   [f%128, fchunk, kd]
            QC = 4   # chunks per DMA quarter
            for q in range(NCH // QC):
                c0 = q * QC
                nc.sync.dma_start(
                    xt32[:, c0:c0 + QC, :],
                    bass.AP(xt_d.tensor, c0 * 128 * IB,
                            [[IB, 128], [128 * IB, QC], [1, IB]]))
                nc.sync.dma_start(
                    w32[:, c0:c0 + QC, :],
                    bass.AP(w_d.tensor, c0 * 128 * KD,
                            [[KD, 128], [128 * KD, QC], [1, KD]]))

            bias_sb = [gemm_in.tile([PC, 1], f32, tag=f"bias{h}", name=f"bias_sb{h}") for h in range(2)]
            for h in range(2):
                nc.sync.dma_start(
                    bias_sb[h][:], bass.AP(b_d.tensor, h * PC, [[1, PC], [0, 1]]))

            # ---- GEMM: actT_local [125,128] x2 = W^T @ x_c^T + b ----
            actT_sb = [acts.tile([PC, IB], f32, tag=f"actT{h}", name=f"actT_sb{h}") for h in range(2)]
            pss = [gemm_ps.tile([PC, IB], f32, tag=f"gps{h}", name=f"gps{h}")
                   for h in range(2)]
            for c in range(NCH):
                for h in range(2):
                    nc.tensor.matmul(
                        pss[h][:], w32[:, c, h * PC:(h + 1) * PC], xt32[:, c, :],
                        start=(c == 0), stop=(c == NCH - 1))
            for h in range(2):
                # PSUM -> SBUF with per-partition bias add
                nc.scalar.activation(actT_sb[h][:], pss[h][:], Ident,
                                     bias=bias_sb[h][:], scale=1.0)

            # local actT in fp16 (shipped through the AllGather) plus an fp32
            # copy of the fp16-rounded values: tensor_scalar scalars must be
            # fp32, and they must bit-match gat16 so the diagonal is 0.
            lact16 = [acts.tile([PC, IB], f16, tag=f"lact16_{h}", name=f"lact16_{h}") for h in range(2)]
            lact32 = [acts.tile([PC, IB], f32, tag=f"lact32_{h}", name=f"lact32_{h}") for h in range(2)]
            for h in range(2):
                nc.vector.tensor_copy(lact16[h][:], actT_sb[h][:])
                nc.vector.tensor_copy(lact32[h][:], lact16[h][:])

            # ---- AllGather actT (fp16) over the 8 cores ----
            ag_in = nc.dram_tensor("ag_in", [2 * PC, IB], f16).ap()
            ag_out = nc.dram_tensor("ag_out", [NCORES * 2 * PC, IB], f16).ap()
            for h in range(2):
                nc.sync.dma_start(ag_in[h * PC:(h + 1) * PC, :], lact16[h][:])
            nc.gpsimd.collective_compute(
                "AllGather", mybir.AluOpType.bypass,
                replica_groups=[list(range(NCORES))],
                ins=[ag_in[:].opt()],
                outs=[ag_out[:].opt()])

            # ---- gather the 8 blocks into [125, 1024] per half ----
            gat16 = [acts.tile([PC, B], f16, tag=f"gat16_{h}", name=f"gat16_{h}") for h in range(2)]
            for h in range(2):
                eng = nc.sync if h == 0 else nc.scalar
                eng.dma_start(
                    gat16[h][:],
                    bass.AP(ag_out.tensor, h * PC * IB,
                            [[IB, PC], [2 * PC * IB, NCORES], [1, IB]]))

            # ---- comb matrices [125, 64] (host-built 0/1 d-summing pattern;
            # the two halves accumulate into one PSUM group, rows k = 0..49;
            # columns 50..63 are zero so the pad rows are initialized)
            combs = [acts.tile([PC, 64], f16, tag=f"comb{h}", name=f"comb{h}") for h in range(2)]
            for h in range(2):
                nc.sync.dma_start(combs[h][:], comb_d[h][:, :])

            # rows: isub*64 + k; rows 50..63 and 114..127 are unused
            feats = outp.tile([128, IB // 2], f32)

            # ---- main loop: 2 i's per group ----
            # ~3/16 of units run fused on ACT (Abs(a_i - in)); the rest on
            # DVE (tensor_scalar subtract at 4x + int16 sign-clear AND).
            ACT_UNITS = {5, 10, 15}
            unit = 0
            for g in range(IB // 2):
                difs = [None, None]
                for isub in range(2):
                    il = 2 * g + isub
                    dt_ = difp.tile([PC, 2 * B], f16, tag=f"dif{isub}",
                                    name=f"dif{isub}_{g}")
                    use_act = (unit % 16) in ACT_UNITS
                    unit += 1
                    if use_act:
                        for h in range(2):
                            nc.scalar.activation(
                                dt_[:, h * B:(h + 1) * B], gat16[h][:], Abs,
                                bias=lact32[h][:, il:il + 1], scale=-1.0)
                    else:
                        for h in range(2):
                            nc.vector.tensor_scalar(
                                out=dt_[:, h * B:(h + 1) * B], in0=gat16[h][:],
                                scalar1=lact32[h][:, il:il + 1], scalar2=None,
                                op0=sub)
                        dti = dt_[:].bitcast(mybir.dt.int16)
                        nc.vector.tensor_scalar(
                            out=dti, in0=dti, scalar1=0x7FFF, scalar2=None,
                            op0=band)
                    difs[isub] = dt_
                l1 = l1p.tile([128, B], f32, tag="l1")
                for isub in range(2):
                    off = isub * 64
                    for jh in range(2):
                        js = slice(jh * 512, (jh + 1) * 512)
                        for h in range(2):
                            seg = h * B + jh * 512
                            nc.tensor.matmul(
                                l1[off:off + 64, js], combs[h][:],
                                difs[isub][:, seg:seg + 512],
                                start=(h == 0), stop=(h == 1))
                nc.scalar.activation(l1[:], l1[:], Exp, scale=-1.0,
                                     accum_out=feats[:, g:g + 1])

            # ---- diagonal eps correction + store ----
            featc = outp.tile([128, IB // 2], f32)
            nc.vector.tensor_scalar(
                out=featc[:], in0=feats[:], scalar1=math.exp(-1.0) - 1.0,
                scalar2=None, op0=mybir.AluOpType.add)
            for isub in range(2):
                nc.sync.dma_start(
                    bass.AP(feat_d, 50 * isub, [[1, 50], [2 * K, IB // 2]]),
                    featc[isub * 64:isub * 64 + 50, :])

    if split_waits:
        _split_excess_waits(nc)
    return nc


_CACHE = {}
TRACE = False


def _in_maps(x, weights, bias):
    xt = np.ascontiguousarray(x.T)  # [F, B]
    combs = []
    for h in range(2):
        c = np.zeros((PC, 64), dtype=np.float16)
        for p in range(PC):
            c[p, p // D + 25 * h] = 1.0
        combs.append(c)
    in_maps = []
    for c in range(NCORES):
        in_maps.append({
            "xt": np.ascontiguousarray(xt[:, c * IB:(c + 1) * IB]),
            "w": weights,
            "bias": bias,
            "comb0": combs[0],
            "comb1": combs[1],
        })
    return in_maps


def kernel(x, weights, bias):
    x = np.ascontiguousarray(x, dtype=np.float32)
    weights = np.ascontiguousarray(weights, dtype=np.float32)
    bias = np.ascontiguousarray(bias, dtype=np.float32)

    if "nc" not in _CACHE:
        _CACHE["nc"] = _build()
    nc = _CACHE["nc"]

    in_maps = _in_maps(x, weights, bias)
    res = run_bass_kernel_spmd(nc, in_maps, list(range(NCORES)), trace=TRACE)
    _CACHE["last_res"] = res
    feats = np.concatenate([res.results[c]["feat"] for c in range(NCORES)],
                           axis=0)  # [B, K]
    return np.concatenate([x, feats.astype(np.float32)], axis=1)



# revision 4
# speedup vs baseline: 1.3107x; 1.2998x over previous
"""MiniBatchDiscrimination kernel for 8 TRN2 NeuronCores.

out = concat([x, f], axis=1) where
  act = (x @ W + b).reshape(B, K, D)
  f[i,k] = sum_j exp(-(sum_d |act[i,k,d]-act[j,k,d]| + (i==j)))

Strategy v2 (no collectives; cores fully independent):
  - Every core receives the FULL x (host-transposed, fp16) and W (fp16,
    padded to 256 cols) and computes the full GEMM actT [250, 1024] on PE
    (fp16 in, fp32 PSUM) -> gat16 [125, 1024] fp16 x2 halves via a bias-add
    Identity activation. This replaces the AllGather (15us constant overhead
    + transfer in the collective cost model) with ~14us of redundant PE work
    that overlaps the input DMAs.
  - A second tiny GEMM over the core's own 128 batch columns (moving operand
    = per-core xtown input) recomputes the core's own activations with the
    identical chunk order, so its fp16 rounding bit-matches gat16 and the
    pairwise diagonal is exactly 0. lact32 = fp32 copy of those fp16 values
    (tensor_scalar scalars must be fp32).
  - Main loop, 2 rows i per group: DVE tensor_scalar with op0=subtract
    (per-partition scalar = lact32 column), op1=abs_max vs 0.0 computes
    |act_i - act_j| fused in ONE 4x-mode pass -> DIFF [125, 2048] fp16.
  - PE matmul vs stationary 0/1 "comb" matrix contracts d (5) within each k
    -> L1 [128 (2 rows x 64), 1024] fp32 in PSUM.
  - ACT exp(scale=-1) with accum_out -> feature column (j-sum fused).
  - diagonal eps: computed exp(0)=1, true exp(-1): add (e^-1 - 1) constant.
Host concatenates x with gathered per-core features.
"""

import math
import numpy as np

import concourse.bass as bass
import concourse.tile as tile
from concourse import mybir
from concourse.bass_utils import run_bass_kernel_spmd
from concourse.vector_clock import ScopedClock, VectorClock

B, F, K, D = 1024, 2048, 50, 5
KD = K * D          # 250
NCORES = 8
IB = B // NCORES    # 128 rows per core
PC = 125            # partition chunk: 25 whole k's of 5 d's
NCH = F // 128      # 16 contraction chunks for the GEMM
WP = 256            # padded W column count (DMA elem runs >= 512B)

f32 = mybir.dt.float32
f16 = mybir.dt.float16


def _patched_drain_and_barrier(self, tick_clock, wait_clock):
    # Walrus in this container rejects the stock tail drain ("Too many sync
    # wait commands"): spread the global-clock waits over one NOP per proc.
    nc = self.nc
    gc = tick_clock.global_clock
    n = len(gc)
    for p in range(n):
        if gc[p] == 0:
            continue
        vec = [0] * n
        vec[p] = gc[p]
        nop = nc.sync.nop(nofuse=True, hint=f"tail_wait_p{p}")
        wait_clock.add_sem_waits(nop.ins, ScopedClock({None: VectorClock(vec)}))
    nc.sync.drain()
    nc.all_engine_barrier()
    assert self.sems is not None
    popped = nc._tile_sem_poison_stack.pop()
    assert popped is self._sem_poison
    nc.clear_and_free_semaphores(list(self.sems.allocated().values()))
    nc.all_engine_barrier()


tile.TileContext._drain_and_barrier = _patched_drain_and_barrier

_ws_ctr = [0]


def _split_excess_waits(nc, max_waits=1):
    """Walrus here allows only one sync-wait per instruction; hoist the rest
    onto same-engine NOPs inserted immediately before (program order on the
    engine preserves semantics)."""
    import bass_rust as _br

    for fn in nc.m.functions:
        new_blocks = []
        for bb in fn.blocks:
            out = []
            changed = False
            for inst in bb.instructions:
                si = inst.sync_info
                if si is not None and len(si.on_wait) > max_waits:
                    waits = list(si.on_wait)
                    for w in waits[:-max_waits]:
                        _ws_ctr[0] += 1
                        nop = mybir.InstNoOp(
                            name=f"WSplit-{_ws_ctr[0]}", ins=[], outs=[])
                        nop.engine = inst.engine
                        nop.sync_info = mybir.SyncInfo(
                            on_wait=[w], on_update=[])
                        out.append(nop)
                    inst.sync_info = mybir.SyncInfo(
                        on_wait=waits[-max_waits:], on_update=list(si.on_update))
                    changed = True
                out.append(inst)
            if changed:
                bb2 = _br.BasicBlock(name=bb.name, instructions=out)
                if bb.IsExit is not None:
                    bb2.IsExit = bb.IsExit
                if bb.IsLoopEntry is not None:
                    bb2.IsLoopEntry = bb.IsLoopEntry
                if bb.IsPredicated is not None:
                    bb2.IsPredicated = bb.IsPredicated
                new_blocks.append(bb2)
            else:
                new_blocks.append(bb)
        fn.blocks = new_blocks


def _build(split_waits=True):
    nc = bass.Bass("TRN2", target_bir_lowering=False, debug=False,
                   num_devices=NCORES)
    xt_d = nc.dram_tensor("xt", [F, B], f16, kind="ExternalInput").ap()
    xo_d = nc.dram_tensor("xo", [F, IB], f16, kind="ExternalInput").ap()
    w_d = nc.dram_tensor("w", [F, WP], f16, kind="ExternalInput").ap()
    b_d = nc.dram_tensor("bias", [KD], f32, kind="ExternalInput").ap()
    comb_d = [nc.dram_tensor(f"comb{h}", [PC, 64], f16, kind="ExternalInput").ap()
              for h in range(2)]
    feat_d = nc.dram_tensor("feat", [IB, K], f32, kind="ExternalOutput")

    sub = mybir.AluOpType.subtract
    absmax = mybir.AluOpType.abs_max
    Exp = mybir.ActivationFunctionType.Exp
    Ident = mybir.ActivationFunctionType.Identity

    with tile.TileContext(nc, num_cores=NCORES) as tc:
        with (
            tc.tile_pool(name="persist", bufs=1) as persist,
            tc.tile_pool(name="gemm_in", bufs=1) as gemm_in,
            tc.tile_pool(name="difp", bufs=3) as difp,
            tc.tile_pool(name="outp", bufs=1) as outp,
        ):
            # ---- load inputs; the big xt DMA is split over 4 engines ----
            xt16 = gemm_in.tile([128, NCH, B], f16)     # xT [f%128, fchunk, j]
            w16 = gemm_in.tile([128, NCH, WP], f16)     # W  [f%128, fchunk, kd]
            xo16 = gemm_in.tile([128, NCH, IB], f16)    # own xT slice
            QC = 4   # chunks per DMA quarter
            nc.scalar.dma_start(
                w16[:],
                bass.AP(w_d.tensor, 0, [[WP, 128], [128 * WP, NCH], [1, WP]]))
            dma_engs = [nc.sync, nc.scalar, nc.gpsimd, nc.gpsimd]
            for q in range(NCH // QC):
                c0 = q * QC
                dma_engs[q].dma_start(
                    xt16[:, c0:c0 + QC, :],
                    bass.AP(xt_d.tensor, c0 * 128 * B,
                            [[B, 128], [128 * B, QC], [1, B]]))
            nc.sync.dma_start(
                xo16[:],
                bass.AP(xo_d.tensor, 0, [[IB, 128], [128 * IB, NCH], [1, IB]]))

            bias_sb = [gemm_in.tile([PC, 1], f32, tag=f"bias{h}",
                                    name=f"bias_sb{h}") for h in range(2)]
            for h in range(2):
                nc.sync.dma_start(
                    bias_sb[h][:], bass.AP(b_d.tensor, h * PC, [[1, PC], [0, 1]]))
            combs = [persist.tile([PC, 64], f16, tag=f"comb{h}",
                                  name=f"comb{h}") for h in range(2)]
            for h in range(2):
                nc.sync.dma_start(combs[h][:], comb_d[h][:, :])

            # ---- full GEMM: actT [250, 1024] = W^T @ x^T + b, fp16 out ----
            gat16 = [persist.tile([PC, B], f16, tag=f"gat16_{h}",
                                  name=f"gat16_{h}") for h in range(2)]
            own16 = [persist.tile([PC, IB], f16, tag=f"own16_{h}",
                                  name=f"own16_{h}") for h in range(2)]
            lact32 = [persist.tile([PC, IB], f32, tag=f"lact32_{h}",
                                   name=f"lact32_{h}") for h in range(2)]
            with tc.tile_pool(name="gemm_ps", bufs=1, space="PSUM") as gemm_ps:
                pss = [[gemm_ps.tile([PC, 512], f32, tag=f"gps{h}{jh}",
                                     name=f"gps{h}{jh}")
                        for jh in range(2)] for h in range(2)]
                pso = [gemm_ps.tile([PC, IB], f32, tag=f"gpso{h}",
                                    name=f"gpso{h}") for h in range(2)]
                for c in range(NCH):
                    for h in range(2):
                        for jh in range(2):
                            nc.tensor.matmul(
                                pss[h][jh][:],
                                w16[:, c, h * PC:(h + 1) * PC],
                                xt16[:, c, jh * 512:(jh + 1) * 512],
                                start=(c == 0), stop=(c == NCH - 1))
                        nc.tensor.matmul(
                            pso[h][:], w16[:, c, h * PC:(h + 1) * PC],
                            xo16[:, c, :],
                            start=(c == 0), stop=(c == NCH - 1))
                for h in range(2):
                    for jh in range(2):
                        nc.scalar.activation(
                            gat16[h][:, jh * 512:(jh + 1) * 512],
                            pss[h][jh][:], Ident,
                            bias=bias_sb[h][:], scale=1.0)
                    nc.scalar.activation(own16[h][:], pso[h][:], Ident,
                                         bias=bias_sb[h][:], scale=1.0)
            for h in range(2):
                nc.vector.tensor_copy(lact32[h][:], own16[h][:])

            # rows: isub*64 + k; rows 50..63 and 114..127 are unused
            feats = outp.tile([128, IB // 2], f32)

            with tc.tile_pool(name="l1", bufs=3, space="PSUM") as l1p:
                # ---- main loop: 2 i's per group ----
                for g in range(IB // 2):
                    difs = [None, None]
                    for isub in range(2):
                        il = 2 * g + isub
                        dt_ = difp.tile([PC, 2 * B], f16, tag=f"dif{isub}",
                                        name=f"dif{isub}_{g}")
                        for h in range(2):
                            nc.vector.tensor_scalar(
                                out=dt_[:, h * B:(h + 1) * B], in0=gat16[h][:],
                                scalar1=lact32[h][:, il:il + 1], scalar2=0.0,
                                op0=sub, op1=absmax)
                        difs[isub] = dt_
                    l1 = l1p.tile([128, B], f32, tag="l1")
                    for isub in range(2):
                        off = isub * 64
                        for jh in range(2):
                            js = slice(jh * 512, (jh + 1) * 512)
                            for h in range(2):
                                seg = h * B + jh * 512
                                nc.tensor.matmul(
                                    l1[off:off + 64, js], combs[h][:],
                                    difs[isub][:, seg:seg + 512],
                                    start=(h == 0), stop=(h == 1))
                    nc.scalar.activation(l1[:], l1[:], Exp, scale=-1.0,
                                         accum_out=feats[:, g:g + 1])

            # ---- diagonal eps correction + store ----
            featc = outp.tile([128, IB // 2], f32)
            nc.vector.tensor_scalar(
                out=featc[:], in0=feats[:], scalar1=math.exp(-1.0) - 1.0,
                scalar2=None, op0=mybir.AluOpType.add)
            for isub in range(2):
                nc.sync.dma_start(
                    bass.AP(feat_d, 50 * isub, [[1, 50], [2 * K, IB // 2]]),
                    featc[isub * 64:isub * 64 + 50, :])

    if split_waits:
        _split_excess_waits(nc)
    return nc


_CACHE = {}
TRACE = False


def _in_maps(x, weights, bias):
    xt16 = np.ascontiguousarray(x.T.astype(np.float16))        # [F, B]
    w16 = np.zeros((F, WP), dtype=np.float16)
    w16[:, :KD] = weights.astype(np.float16)
    combs = []
    for h in range(2):
        c = np.zeros((PC, 64), dtype=np.float16)
        for p in range(PC):
            c[p, p // D + 25 * h] = 1.0
        combs.append(c)
    in_maps = []
    for c in range(NCORES):
        in_maps.append({
            "xt": xt16,
            "xo": np.ascontiguousarray(xt16[:, c * IB:(c + 1) * IB]),
            "w": w16,
            "bias": bias.astype(np.float32),
            "comb0": combs[0],
            "comb1": combs[1],
        })
    return in_maps


def kernel(x, weights, bias):
    x = np.ascontiguousarray(x, dtype=np.float32)
    weights = np.ascontiguousarray(weights, dtype=np.float32)
    bias = np.ascontiguousarray(bias, dtype=np.float32)

    if "nc" not in _CACHE:
        _CACHE["nc"] = _build()
    nc = _CACHE["nc"]

    in_maps = _in_maps(x, weights, bias)
    res = run_bass_kernel_spmd(nc, in_maps, list(range(NCORES)), trace=TRACE)
    _CACHE["last_res"] = res
    feats = np.concatenate([res.results[c]["feat"] for c in range(NCORES)],
                           axis=0)  # [B, K]
    return np.concatenate([x, feats.astype(np.float32)], axis=1)
